# revision 1
# baseline (speedup 1.0000x reference)
"""AttentionalCopula Trainium2 kernel.

Data-parallel over batch: 8 NeuronCores, 2 batch elements per core.
All activations transposed-friendly layouts prepared on host; device does
matmuls in float32r (full PE rate, ~tf32 precision), fp32 vector ops.

Self-contained: hardcodes shapes from the problem spec.
"""
import math
import sys

import numpy as np

sys.path.insert(0, "/opt/trn_rl_repo")

import concourse.bass as bass  # noqa: E402
import concourse.bacc as bacc  # noqa: E402
import concourse.tile as tile  # noqa: E402
import concourse.mybir as mybir  # noqa: E402
from contextlib import ExitStack  # noqa: E402

F32 = mybir.dt.float32
F32R = mybir.dt.float32r
AF = mybir.ActivationFunctionType
ALU = mybir.AluOpType

B, D, NH, NS, NT = 16, 256, 512, 8, 32
NV = NS * NT
L, H, A = 4, 8, 64
HA = H * A
M = 512
R = 128
W = NH + NV
EPS = 1e-5
SCALE = A ** -0.5
NCORES = 8
EPC = B // NCORES  # elems per core

_BUILD_CACHE = {}


def ts(i, n):
    return slice(i * n, (i + 1) * n)


_DEBUG = False
_NPHASE = 99  # debug bisect: 1=ds only, 2=+keys/vals, 3=+attention, 4=+ln/ff, 99=full


def _build(use_ff_bias, use_de_bias, ln_affine):
    nc = bacc.Bacc(None, target_bir_lowering=False)

    def P(name, shape, out=False, dt=F32):
        return nc.declare_dram_parameter(name, shape, dt, isOutput=out)

    kiT_d = P("kiT", (EPC, 258, W), dt=F32R)
    kw_d = P("kwp", (L, 258, HA), dt=F32R)
    vw_d = P("vwp", (L, 258, HA), dt=F32R)
    ds_d = P("dswp", (258, HA), dt=F32R)
    f1_d = P("ffw1", (L, 513, M), dt=F32R)
    f2_d = P("ffw2", (L, 513, M), dt=F32R)
    f3_d = P("ffw3", (L, 513, HA), dt=F32R)
    dew_d = P("dew", (HA, R), dt=F32R)
    deb_d = P("deb", (1, R), dt=F32R)
    mask_d = P("maskmul", (128, 128))
    oh_d = P("onehot", (EPC, 2, 128, R))
    id_d = P("ident", (128, 128), dt=F32R)
    wv_d = P("wv0", (128, 1))
    onesr_d = P("onesrow", (1, W), dt=F32R)
    onesc_d = P("onescol", (128, 1), dt=F32R)
    vones_d = P("vones", (128, 96), dt=F32R)
    if ln_affine:
        lnp_d = P("lnp", (L, 4, HA))
    out_d = P("out", (1, EPC), out=True)
    if _DEBUG:
        dbg_keys_d = P("dbg_keys", (128, 4, W), out=True)
        dbg_vals_d = P("dbg_vals", (128, 6, 8, 66), out=True)
        dbg_exp_d = P("dbg_exp", (128, 1536), out=True)
        dbg_att0_d = P("dbg_att0", (128, 2, HA), out=True)
        dbg_attr_d = P("dbg_attr", (128, 2, HA), out=True)
        dbg_att1_d = P("dbg_att1", (128, 2, HA), out=True)
        dbg_lg_d = P("dbg_lg", (2, 128, R), out=True)

    with tile.TileContext(nc) as tc, ExitStack() as ctx:
        const = ctx.enter_context(tc.tile_pool(name="const", bufs=1))
        kpool = ctx.enter_context(tc.tile_pool(name="kvw", bufs=2))
        fpool = ctx.enter_context(tc.tile_pool(name="ffw", bufs=2))
        iopool = ctx.enter_context(tc.tile_pool(name="io", bufs=2))
        kvpool = ctx.enter_context(tc.tile_pool(name="keys", bufs=1))
        epool = ctx.enter_context(tc.tile_pool(name="exp", bufs=3))
        apool = ctx.enter_context(tc.tile_pool(name="att", bufs=5))
        tpool = ctx.enter_context(tc.tile_pool(name="attT", bufs=3))
        ftpool = ctx.enter_context(tc.tile_pool(name="ffT", bufs=2))
        spool = ctx.enter_context(tc.tile_pool(name="small", bufs=4))
        ps_s = ctx.enter_context(tc.tile_pool(name="ps_s", bufs=1, space="PSUM"))
        ps_b = ctx.enter_context(tc.tile_pool(name="ps_b", bufs=4, space="PSUM"))
        ps_a = ctx.enter_context(tc.tile_pool(name="ps_a", bufs=1, space="PSUM"))

        dma = nc.sync.dma_start

        # ---- constants ----
        ident = const.tile([128, 128], F32R, tag="ident")
        dma(ident[:], id_d.ap())
        maskm = const.tile([128, 128], F32, tag="maskm")
        dma(maskm[:], mask_d.ap())
        onehot_t = const.tile([128, EPC * 2, R], F32, tag="onehot")
        for e in range(EPC):
            for vt in range(2):
                dma(onehot_t[:, e * 2 + vt, :], oh_d.ap()[e, vt])
        wv0 = const.tile([128, 1], F32, tag="wv0")
        dma(wv0[:], wv_d.ap())
        ones_row = const.tile([1, W], F32R, tag="ones_row")
        dma(ones_row[:], onesr_d.ap())
        ones_col = const.tile([128, 1], F32R, tag="ones_col")
        dma(ones_col[:], onesc_d.ap())
        dsw_t = const.tile([128, 2, HA], F32R, tag="dsw")
        dma(dsw_t[:], ds_d.ap()[0:256].rearrange("(a p) n -> p a n", p=128))
        dsu_t = const.tile([2, HA], F32R, tag="dsu")
        dma(dsu_t[:], ds_d.ap()[256:258])
        dew_t = const.tile([128, 4, R], F32R, tag="dew")
        dma(dew_t[:], dew_d.ap().rearrange("(a p) n -> p a n", p=128))
        deb_t = const.tile([1, R], F32R, tag="deb")
        dma(deb_t[:], deb_d.ap())
        if use_ff_bias:
            ffb_t = const.tile([12, M], F32R, tag="ffb")
            for mi, fd in enumerate((f1_d, f2_d, f3_d)):
                for l in range(L):
                    dma(ffb_t[mi * 4 + l: mi * 4 + l + 1, :], fd.ap()[l, 512:513, :])
        if ln_affine:
            lnp_t = const.tile([16, HA], F32, tag="lnp")
            for l in range(L):
                for j in range(4):
                    dma(lnp_t[l * 4 + j: l * 4 + j + 1, :], lnp_d.ap()[l, j: j + 1, :])
        res_sb = const.tile([1, EPC], F32, tag="res")
        if _NPHASE < 99:
            nc.gpsimd.memset(res_sb[:], 0.0)
        keysT = const.tile([128, 4, W], F32R, tag="keys")
        vals = const.tile([128, 6, 8, 66], F32R, tag="vals")
        dma(vals[:, :, :, 64:66], vones_d.ap().rearrange("p (a b c) -> p a b c", a=6, b=8))
        eps_t = const.tile([128, 1], F32, tag="eps")
        nc.gpsimd.memset(eps_t[:], EPS)
        sc8_t = const.tile([128, 1], F32, tag="sc8")
        nc.gpsimd.memset(sc8_t[:], SCALE)
        neg1_t = const.tile([1, 1], F32, tag="neg1")
        nc.gpsimd.memset(neg1_t[:], -1.0)
        fbias_t = const.tile([1, 1], F32, tag="fbias")
        nc.gpsimd.memset(fbias_t[:], -(NV - 1) * math.log(R))

        evac_ctr = [0]

        def evac(out_ap, in_ap):
            # PSUM->SBUF copies: 2/3 on DVE, 1/3 on ACT (ACT is exp-bound)
            if evac_ctr[0] % 3 < 2:
                nc.vector.tensor_copy(out_ap, in_ap)
            else:
                nc.scalar.copy(out_ap, in_ap)
            evac_ctr[0] += 1

        def mm(ps_ap, chunks, dt=F32R):
            n = len(chunks)
            for i, (lh, rh) in enumerate(chunks):
                nc.tensor.matmul(ps_ap, lh, rh,
                                 start=(i == 0), stop=(i == n - 1))

        def ln_apply(out_ap, in_ap, l, which, vt, small):
            """LayerNorm along free dim (HA) of [128, HA] tile."""
            st6 = small.tile([128, 6], F32, tag="st6")
            nc.vector.bn_stats(st6[:], in_ap)
            mv = small.tile([128, 2], F32, tag="mv")
            nc.vector.bn_aggr(mv[:], st6[:])
            sd = small.tile([128, 1], F32, tag="sd")
            nc.scalar.activation(sd[:], mv[:, 1:2], AF.Sqrt, bias=eps_t[:, 0:1])
            rs = small.tile([128, 1], F32, tag="rs")
            nc.vector.reciprocal(rs[:], sd[:])
            nb = small.tile([128, 1], F32, tag="nb")
            nc.vector.tensor_scalar(nb[:], mv[:, 0:1], rs[:, 0:1], -1.0,
                                    op0=ALU.mult, op1=ALU.mult)
            if not ln_affine:
                nc.scalar.activation(out_ap, in_ap, AF.Identity,
                                     bias=nb[:, 0:1], scale=rs[:, 0:1])
            else:
                t0 = small.tile([128, HA], F32, tag="lnt0")
                nc.scalar.activation(t0[:], in_ap, AF.Identity,
                                     bias=nb[:, 0:1], scale=rs[:, 0:1])
                gb = small.tile([128, HA], F32, tag="lngb")
                gi = l * 4 + (0 if which == 1 else 2)
                nc.gpsimd.partition_broadcast(gb[:], lnp_t[gi: gi + 1, :])
                nc.vector.tensor_mul(t0[:], t0[:], gb[:])
                bi = gi + 1
                nc.gpsimd.partition_broadcast(gb[:], lnp_t[bi: bi + 1, :])
                nc.vector.tensor_add(out_ap, t0[:], gb[:])

        # ================== per batch element ==================
        for e in range(EPC):
            ki0 = iopool.tile([128, W], F32R, tag="ki0")
            ki1 = iopool.tile([128, W], F32R, tag="ki1")
            kiu = iopool.tile([2, W], F32R, tag="kiu")
            dma(ki0[:], kiT_d.ap()[e, 0:128])
            dma(ki1[:], kiT_d.ap()[e, 128:256])
            dma(kiu[:], kiT_d.ap()[e, 256:258])
            kich = [ki0, ki1]

            # ---- initial att (natural [v,ha]) and attT ([ha,v]) ----
            att = apool.tile([128, 2, HA], F32R, tag="att")
            for vt in range(2):
                ps = ps_b.tile([128, 512], F32, tag="psb")
                mm(ps[:], [(ki0[:, 512 + vt * 128: 512 + (vt + 1) * 128], dsw_t[:, 0, :]),
                           (ki1[:, 512 + vt * 128: 512 + (vt + 1) * 128], dsw_t[:, 1, :]),
                           (kiu[:, 512 + vt * 128: 512 + (vt + 1) * 128], dsu_t[:, :])])
                evac(att[:, vt, :], ps[:])
            attT = tpool.tile([128, 4, NV], F32R, tag="attT")
            for t in range(4):
                ps = ps_b.tile([128, 512], F32, tag="psb")
                mm(ps[:, 0:NV], [(dsw_t[:, 0, ts(t, 128)], ki0[:, 512:768]),
                                 (dsw_t[:, 1, ts(t, 128)], ki1[:, 512:768]),
                                 (dsu_t[:, ts(t, 128)], kiu[:, 512:768])])
                evac(attT[:, t, :], ps[:, 0:NV])

            # ================== layers ==================
            for l in range(L if _NPHASE in (5, 99) else (1 if _NPHASE >= 2 else 0)):
                kw_t = kpool.tile([128, 2, HA], F32R, tag="kw")
                dma(kw_t[:], kw_d.ap()[l, 0:256].rearrange("(a p) n -> p a n", p=128))
                vw_t = kpool.tile([128, 2, HA], F32R, tag="vw")
                dma(vw_t[:], vw_d.ap()[l, 0:256].rearrange("(a p) n -> p a n", p=128))
                kvu_t = kpool.tile([2, 2, HA], F32R, tag="kvu")
                dma(kvu_t[:, 0, :], kw_d.ap()[l, 256:258])
                dma(kvu_t[:, 1, :], vw_d.ap()[l, 256:258])
                ffw1_t = fpool.tile([128, 4, M], F32R, tag="f1")
                dma(ffw1_t[:], f1_d.ap()[l, 0:512].rearrange("(a p) n -> p a n", p=128))
                ffw2_t = fpool.tile([128, 4, M], F32R, tag="f2")
                dma(ffw2_t[:], f2_d.ap()[l, 0:512].rearrange("(a p) n -> p a n", p=128))
                ffw3_t = fpool.tile([128, 4, HA], F32R, tag="f3")
                dma(ffw3_t[:], f3_d.ap()[l, 0:512].rearrange("(a p) n -> p a n", p=128))

                # ---- keysT [ha, w] ----
                for t in range(4):
                    for (wlo, wn) in ((0, 512), (512, 256)):
                        ps = ps_b.tile([128, 512], F32, tag="psb")
                        mm(ps[:, 0:wn],
                           [(kw_t[:, 0, ts(t, 128)], ki0[:, wlo:wlo + wn]),
                            (kw_t[:, 1, ts(t, 128)], ki1[:, wlo:wlo + wn]),
                            (kvu_t[:, 0, ts(t, 128)], kiu[:, wlo:wlo + wn])])
                        evac(keysT[:, t, wlo:wlo + wn], ps[:, 0:wn])

                # ---- vals [w, (h,a)] with ones column per head ----
                for wt in range(6):
                    ps = ps_b.tile([128, 512], F32, tag="psb")
                    mm(ps[:], [(ki0[:, ts(wt, 128)], vw_t[:, 0, :]),
                               (ki1[:, ts(wt, 128)], vw_t[:, 1, :]),
                               (kiu[:, ts(wt, 128)], kvu_t[:, 1, :])])
                    evac(vals[:, wt, :, 0:64], ps[:].rearrange("p (h a) -> p h a", h=8))

                # ---- attention ----
                att_res = apool.tile([128, 2, HA], F32R, tag="att")
                for h in range(H if _NPHASE >= 3 else 0):
                    t, base = h // 2, (h % 2) * 64
                    ps_st = ps_s.tile([128, 1536], F32, tag="s")
                    for wt in range(6):
                        nc.tensor.matmul(
                            ps_st[:, ts(wt, 256)],
                            keysT[base:base + 64, t, ts(wt, 128)],
                            attT[base:base + 64, t, :],
                            start=True, stop=True)
                    expT = epool.tile([128, 1536], F32R, tag="exp")
                    nc.scalar.activation(expT[:], ps_st[:], AF.Exp, scale=sc8_t[:, 0:1])
                    nc.vector.tensor_mul(expT[:, 1024:1152], expT[:, 1024:1152], maskm[:])
                    nc.vector.tensor_mul(expT[:, 1408:1536], expT[:, 1408:1536], maskm[:])
                    nc.vector.tensor_scalar_mul(expT[:, 1280:1408],
                                                expT[:, 1280:1408], 0.0)
                    ps_at = ps_a.tile([66, 256], F32, tag="a")
                    for wt in range(6):
                        nc.tensor.matmul(ps_at[:], vals[:, wt, h, :],
                                         expT[:, ts(wt, 256)],
                                         start=(wt == 0), stop=(wt == 5))
                    aT_s = spool.tile([66, 256], F32R, tag="aTs")
                    evac(aT_s[:], ps_at[:])
                    if _DEBUG and e == 0 and l == 0 and h == 0:
                        dma(dbg_exp_d.ap()[:], expT[:].bitcast(F32))
                    ps_tr = ps_b.tile([128, 512], F32R, tag="psb")
                    rec = spool.tile([128, 2], F32, tag="rec")
                    for half in range(2):
                        nc.tensor.transpose(ps_tr[:, half * 66:half * 66 + 66],
                                            aT_s[:, ts(half, 128)], ident[0:66, 0:66])
                    for half in range(2):
                        nc.vector.reciprocal(rec[:, half:half + 1],
                                             ps_tr[:, half * 66 + 64:half * 66 + 65])
                    for half in range(2):
                        nc.vector.scalar_tensor_tensor(
                            att_res[:, half, ts(h, 64)],
                            ps_tr[:, half * 66:half * 66 + 64],
                            rec[:, half:half + 1],
                            att[:, half, ts(h, 64)],
                            op0=ALU.mult, op1=ALU.add)

                if _DEBUG and e == 0 and l == 0:
                    dma(dbg_keys_d.ap()[:], keysT[:].bitcast(F32))
                    dma(dbg_vals_d.ap()[:], vals[:].bitcast(F32))
                    dma(dbg_att0_d.ap()[:], att[:].bitcast(F32))
                    dma(dbg_attr_d.ap()[:], att_res[:].bitcast(F32))
                # ---- LN1 ----
                if _NPHASE < 4:
                    continue
                att1 = apool.tile([128, 2, HA], F32R, tag="att")
                for vt in range(2):
                    ln_apply(att1[:, vt, :], att_res[:, vt, :], l, 1, vt, spool)
                if _DEBUG and e == 0 and l == 0:
                    dma(dbg_att1_d.ap()[:], att1[:].bitcast(F32))
                att1T = tpool.tile([128, 4, NV], F32R, tag="attT")
                for c in range(4):
                    ps_tr = ps_b.tile([128, 512], F32R, tag="psb")
                    for vt in range(2):
                        nc.tensor.transpose(ps_tr[:, ts(vt, 128)],
                                            att1[:, vt, ts(c, 128)], ident[:])
                    evac(att1T[:, c, :], ps_tr[:, 0:256])

                # ---- FF ----
                ff1T = ftpool.tile([128, 4, NV], F32R, tag="ffT")
                for mt in range(4):
                    ps = ps_b.tile([128, 512], F32, tag="psb")
                    ch = [(ffw1_t[:, c, ts(mt, 128)], att1T[:, c, :]) for c in range(4)]
                    if use_ff_bias:
                        ch.append((ffb_t[l:l + 1, ts(mt, 128)], ones_row[:, 0:NV]))
                    mm(ps[:, 0:NV], ch)
                    nc.vector.tensor_scalar_max(ff1T[:, mt, :], ps[:, 0:NV], 0.0)
                ff2T = ftpool.tile([128, 4, NV], F32R, tag="ffT")
                for mt in range(4):
                    ps = ps_b.tile([128, 512], F32, tag="psb")
                    ch = [(ffw2_t[:, c, ts(mt, 128)], ff1T[:, c, :]) for c in range(4)]
                    if use_ff_bias:
                        ch.append((ffb_t[4 + l:5 + l, ts(mt, 128)], ones_row[:, 0:NV]))
                    mm(ps[:, 0:NV], ch)
                    nc.vector.tensor_scalar_max(ff2T[:, mt, :], ps[:, 0:NV], 0.0)
                att2_res = apool.tile([128, 2, HA], F32R, tag="att")
                for c in range(4):
                    ps3 = ps_b.tile([128, 512], F32, tag="psb")
                    ch = [(ffw3_t[:, k, ts(c, 128)], ff2T[:, k, :]) for k in range(4)]
                    if use_ff_bias:
                        ch.append((ffb_t[8 + l:9 + l, ts(c, 128)], ones_row[:, 0:NV]))
                    mm(ps3[:, 0:NV], ch)
                    f3s = ftpool.tile([128, NV], F32R, tag="f3s")
                    evac(f3s[:], ps3[:, 0:NV])
                    ps_tr = ps_b.tile([128, 512], F32R, tag="psb")
                    for vt in range(2):
                        nc.tensor.transpose(ps_tr[:, ts(vt, 128)],
                                            f3s[:, ts(vt, 128)], ident[:])
                    for vt in range(2):
                        nc.vector.tensor_add(att2_res[:, vt, ts(c, 128)],
                                             ps_tr[:, ts(vt, 128)],
                                             att1[:, vt, ts(c, 128)])

                # ---- LN2 ----
                att2 = apool.tile([128, 2, HA], F32R, tag="att")
                for vt in range(2):
                    ln_apply(att2[:, vt, :], att2_res[:, vt, :], l, 2, vt, spool)
                att2T = tpool.tile([128, 4, NV], F32R, tag="attT")
                for c in range(4):
                    ps_tr = ps_b.tile([128, 512], F32R, tag="psb")
                    for vt in range(2):
                        nc.tensor.transpose(ps_tr[:, ts(vt, 128)],
                                            att2[:, vt, ts(c, 128)], ident[:])
                    evac(att2T[:, c, :], ps_tr[:, 0:256])
                att, attT = att2, att2T

            # ================== loss head ==================
            if _NPHASE in (5,) or _NPHASE < 4:
                continue
            if _NPHASE < 99 and _NPHASE >= 10:
                pass
            q = spool.tile([128, 2], F32R, tag="q")
            for vt in range(2):
                ps = ps_b.tile([128, 512], F32, tag="psb")
                ch = [(attT[:, c, ts(vt, 128)], dew_t[:, c, :]) for c in range(4)]
                if use_de_bias:
                    ch.append((ones_row[0:1, ts(vt, 128)], deb_t[:, :]))
                mm(ps[:, 0:R], ch)
                lg = spool.tile([128, R], F32, tag="lg")
                evac(lg[:], ps[:, 0:R])
                if _DEBUG and e == 0:
                    dma(dbg_lg_d.ap()[vt], lg[:])
                if _NPHASE < 11:
                    continue
                scr = spool.tile([128, R], F32, tag="scr")
                se = spool.tile([128, 1], F32, tag="se")
                nc.scalar.activation(scr[:], lg[:], AF.Exp, accum_out=se[:])
                if _NPHASE < 12:
                    continue
                lse = spool.tile([128, 1], F32, tag="lse")
                nc.scalar.activation(lse[:], se[:], AF.Ln)
                if _NPHASE < 13:
                    continue
                pick = spool.tile([128, 1], F32, tag="pick")
                nc.vector.tensor_mul(scr[:], lg[:], onehot_t[:, e * 2 + vt, :])
                nc.vector.tensor_reduce(pick[:], scr[:], mybir.AxisListType.X,
                                        ALU.add)
                if _NPHASE < 14:
                    continue
                nc.vector.scalar_tensor_tensor(
                    q[:, vt:vt + 1], lse[:], -1.0, pick[:],
                    op0=ALU.mult, op1=ALU.add)
            if _NPHASE < 15:
                continue
            nc.vector.tensor_mul(q[:, 0:1], q[:, 0:1], wv0[:])
            ps_l = ps_a.tile([66, 256], F32, tag="a")
            nc.tensor.matmul(ps_l[0:1, 0:2], ones_col[:], q[:, 0:2],
                             start=True, stop=True)
            tot = spool.tile([1, 1], F32, tag="tot")
            nc.vector.tensor_reduce(tot[:], ps_l[0:1, 0:2], mybir.AxisListType.X,
                                    ALU.add)
            nc.scalar.activation(res_sb[0:1, e:e + 1], tot[0:1, 0:1], AF.Identity,
                                 scale=neg1_t[0:1, 0:1], bias=fbias_t[0:1, 0:1])
        dma(out_d.ap()[0:1, :], res_sb[:])

    nc.finalize()
    return nc


def _prep_inputs(inputs):
    hist_encoded = np.asarray(inputs["hist_encoded"], np.float32)
    hist_true_u = np.asarray(inputs["hist_true_u"], np.float32)
    pred_encoded = np.asarray(inputs["pred_encoded"], np.float32)
    pred_true_u = np.asarray(inputs["pred_true_u"], np.float32)
    key_w = np.asarray(inputs["key_w"], np.float32)
    key_b = np.asarray(inputs["key_b"], np.float32)
    val_w = np.asarray(inputs["val_w"], np.float32)
    val_b = np.asarray(inputs["val_b"], np.float32)
    ds_w = np.asarray(inputs["ds_w"], np.float32)
    ds_b = np.asarray(inputs["ds_b"], np.float32)
    ff_w1 = np.asarray(inputs["ff_w1"], np.float32)
    ff_b1 = np.asarray(inputs["ff_b1"], np.float32)
    ff_w2 = np.asarray(inputs["ff_w2"], np.float32)
    ff_b2 = np.asarray(inputs["ff_b2"], np.float32)
    ff_w3 = np.asarray(inputs["ff_w3"], np.float32)
    ff_b3 = np.asarray(inputs["ff_b3"], np.float32)
    de_w = np.asarray(inputs["de_w"], np.float32)
    de_b = np.asarray(inputs["de_b"], np.float32)
    ln1_g = np.asarray(inputs["ln1_g"], np.float32)
    ln1_b = np.asarray(inputs["ln1_b"], np.float32)
    ln2_g = np.asarray(inputs["ln2_g"], np.float32)
    ln2_b = np.asarray(inputs["ln2_b"], np.float32)

    # kiT per batch elem: [258, W]
    enc = np.concatenate([hist_encoded, pred_encoded], axis=1)  # [B, W, D]
    u = np.concatenate([hist_true_u, pred_true_u], axis=1)      # [B, W]
    kiT = np.empty((B, 258, W), np.float32)
    kiT[:, 0:256, :] = enc.transpose(0, 2, 1)
    kiT[:, 256, :] = u
    kiT[:, 257, :] = 1.0

    def pack_kv(wt, bt):  # [L,H,257,A],[L,H,A] -> [L,258,HA]
        p = np.empty((L, 258, HA), np.float32)
        p[:, 0:257, :] = wt.transpose(0, 2, 1, 3).reshape(L, 257, HA)
        p[:, 257, :] = bt.reshape(L, HA)
        return p

    kwp = pack_kv(key_w, key_b)
    vwp = pack_kv(val_w, val_b)

    dswp = np.zeros((258, HA), np.float32)
    dswp[0:256] = ds_w
    dswp[257] = ds_b

    def pack_ff(wt, bt, n):
        p = np.empty((L, 513, n), np.float32)
        p[:, 0:512, :] = wt
        p[:, 512, :] = bt
        return p

    ffw1 = pack_ff(ff_w1, ff_b1, M)
    ffw2 = pack_ff(ff_w2, ff_b2, M)
    ffw3 = pack_ff(ff_w3, ff_b3, HA)

    rho = np.arange(128)[:, None]
    vv = np.arange(128)[None, :]
    maskmul = (vv > rho).astype(np.float32)  # 0 where v <= rho (masked)

    tgt = np.clip(np.floor(pred_true_u * R).astype(np.int64), 0, R - 1)  # [B, NV]
    onehot = np.zeros((B, 2, 128, R), np.float32)
    for vt in range(2):
        idx = tgt[:, vt * 128:(vt + 1) * 128]
        onehot[np.arange(B)[:, None], vt, np.arange(128)[None, :], idx] = 1.0
    onehot[:, 0, 0, :] = 0.0  # exclude v=0

    ident = np.eye(128, dtype=np.float32)
    wv0 = np.ones((128, 1), np.float32)
    wv0[0, 0] = 0.0

    use_ff_bias = bool(np.any(ff_b1) or np.any(ff_b2) or np.any(ff_b3))
    use_de_bias = bool(np.any(de_b))
    ln_affine = bool(np.any(ln1_g != 1.0) or np.any(ln1_b) or
                     np.any(ln2_g != 1.0) or np.any(ln2_b))
    lnp = np.stack([ln1_g, ln1_b, ln2_g, ln2_b], axis=1)  # [L,4,HA]

    shared = {
        "kwp": kwp, "vwp": vwp, "dswp": dswp,
        "ffw1": ffw1, "ffw2": ffw2, "ffw3": ffw3,
        "dew": de_w, "deb": de_b.reshape(1, R),
        "maskmul": maskmul, "ident": ident, "wv0": wv0,
        "onesrow": np.ones((1, W), np.float32),
        "onescol": np.ones((128, 1), np.float32),
        "vones": np.tile(np.array([1.0, 0.0], np.float32), 48).reshape(1, 96).repeat(128, 0),
    }
    if ln_affine:
        shared["lnp"] = lnp
    in_maps = []
    for c in range(NCORES):
        m = dict(shared)
        m["kiT"] = kiT[c * EPC:(c + 1) * EPC]
        m["onehot"] = onehot[c * EPC:(c + 1) * EPC]
        in_maps.append(m)
    return in_maps, (use_ff_bias, use_de_bias, ln_affine)


def _get_nc(flags):
    if flags not in _BUILD_CACHE:
        _BUILD_CACHE[flags] = _build(*flags)
    return _BUILD_CACHE[flags]


def _run(inputs, trace=False):
    from concourse.bass_utils import run_bass_kernel_spmd
    in_maps, flags = _prep_inputs(inputs)
    nc = _get_nc(flags)
    res = run_bass_kernel_spmd(nc, in_maps, list(range(NCORES)), trace=trace)
    out = np.concatenate([res.results[c]["out"].reshape(EPC)
                          for c in range(NCORES)])
    return out.astype(np.float32), res


def kernel(**inputs) -> np.ndarray:
    out, _ = _run(inputs, trace=False)
    return out



# revision 24
# speedup vs baseline: 1.3075x; 1.3075x over previous
"""AttentionalCopula Trainium2 kernel (v2).

Data-parallel over batch: 8 NeuronCores x 2 batch elements per core, with the
two elements processed together so shared-weight matmuls stream 512-col tiles
and the PE stays busy.

Key structure per layer:
  - keys/vals creation: f32r matmuls into [128,1536] PSUM tiles, plain evacs
    split between DVE and ACT.
  - attention: 16 (elem, head) streams, software-pipelined:
      scores (PE, f32r) -> exp (ACT, bf16 out) -> mask (DVE/Pool, bf16)
      -> transposed-AV (PE, bf16: exp stationary, vals moving) which lands the
      head output directly in [v, ha] orientation in PSUM; normalization +
      residual-add read the PSUM directly (no per-head transposes).
  - LayerNorm without ACT table swaps: 1/sqrt(var+eps) = exp(-0.5*ln(var+eps))
    so the ACT engine only ever uses the exp/ln/identity/relu table.
  - FF: 512-col matmuls over both elements at once.

Self-contained: hardcodes shapes from the problem spec.
"""
import math
import sys

import numpy as np

sys.path.insert(0, "/opt/trn_rl_repo")

import concourse.bass as bass  # noqa: E402
import concourse.bacc as bacc  # noqa: E402
import concourse.tile as tile  # noqa: E402
import concourse.mybir as mybir  # noqa: E402
from contextlib import ExitStack  # noqa: E402

F32 = mybir.dt.float32
F32R = mybir.dt.float32r
BF16 = mybir.dt.bfloat16
AF = mybir.ActivationFunctionType
ALU = mybir.AluOpType

B, D, NH, NS, NT = 16, 256, 512, 8, 32
NV = NS * NT          # 256
L, H, A = 4, 8, 64
HA = H * A            # 512
M = 512
R = 128
W = NH + NV           # 768
W2 = 2 * W            # 1536
EPS = 1e-5
SCALE = A ** -0.5
NCORES = 8
EPC = B // NCORES     # 2

_BUILD_CACHE = {}
_LN_SQRT = True  # use ACT Sqrt (table swap) instead of exp(-0.5*ln(var))
_DEBUG = False


def ts(i, n):
    return slice(i * n, (i + 1) * n)


def _build(use_ff_bias, use_de_bias, ln_affine, use_ds_bias, use_kv_bias):
    nc = bacc.Bacc(None, target_bir_lowering=False)

    def P(name, shape, out=False, dt=F32):
        return nc.declare_dram_parameter(name, shape, dt, isOutput=out)

    ki_d = P("kiT", (258, W2), dt=F32R)
    dsw_d = P("dswp", (256, HA), dt=F32R)
    kw_d = P("kwp", (L, 256, HA), dt=F32R)
    vw_d = P("vwp", (L, 256, HA), dt=F32R)
    kvu_d = P("kvup", (L, 2, 2, HA), dt=F32R)  # [kv][u-row, ones-row]
    f1_d = P("ffw1", (L, 512, M), dt=F32R)
    f2_d = P("ffw2", (L, M, M), dt=F32R)
    f3_d = P("ffw3", (L, M, HA), dt=F32R)
    dew_d = P("dewp", (512, 256), dt=F32R)     # R padded 128->256
    mask_d = P("maskm", (128, 128), dt=BF16)
    oh_d = P("onehot", (128, 4, R))
    id_d = P("ident", (128, 128), dt=F32R)
    wv_d = P("wv4", (128, 4))
    vone_d = P("vones", (128, 12, 8, 2), dt=BF16)
    onec_d = P("onescol", (128, 1), dt=F32R)
    if use_ds_bias:
        dsb_d = P("dsb", (2, HA), dt=F32R)  # row0 zeros, row1 = ds_b
    if use_ff_bias:
        fbc_d = P("ffbc", (L, 128, 12))
    if use_de_bias:
        debc_d = P("debc", (128, 256))
    if ln_affine:
        lnbc_d = P("lnbc", (L, 128, 4, HA))
    out_d = P("out", (1, EPC), out=True)
    if _DEBUG:
        dbg_att0_d = P("dbg_att0", (128, 4, HA), out=True)
        dbg_attT0_d = P("dbg_attT0", (128, 4, 2 * NV), out=True)
        dbg_keysT_d = P("dbg_keysT", (128, 4, W2), out=True)
        dbg_vals_d = P("dbg_vals", (128, 12, 8, 66), out=True, dt=BF16)
        dbg_exp_d = P("dbg_exp", (128, W2), out=True, dt=BF16)
        dbg_attres_d = P("dbg_attres", (128, 4, HA), out=True)
        dbg_att1_d = P("dbg_att1", (128, 4, HA), out=True)
        dbg_attL_d = P("dbg_attL", (128, 4, HA), out=True)
        dbg_lg_d = P("dbg_lg", (128, 256), out=True)

    with tile.TileContext(nc) as tc, ExitStack() as ctx:
        const = ctx.enter_context(tc.tile_pool(name="const", bufs=1))
        kvw = ctx.enter_context(tc.tile_pool(name="kvw", bufs=2))
        ffw = ctx.enter_context(tc.tile_pool(name="ffw", bufs=1))
        ep = ctx.enter_context(tc.tile_pool(name="ep", bufs=2))
        atp = ctx.enter_context(tc.tile_pool(name="atp", bufs=3))
        atT = ctx.enter_context(tc.tile_pool(name="atT", bufs=2))
        ffp = ctx.enter_context(tc.tile_pool(name="ffp", bufs=2))
        sm = ctx.enter_context(tc.tile_pool(name="sm", bufs=4))
        ps = ctx.enter_context(tc.tile_pool(name="ps", bufs=1, space="PSUM"))

        dma = nc.sync.dma_start

        # ---- inputs/constants ----
        ki0 = const.tile([128, W2], F32R, tag="ki0")
        dma(ki0[:], ki_d.ap()[0:128])
        ki1 = const.tile([128, W2], F32R, tag="ki1")
        dma(ki1[:], ki_d.ap()[128:256])
        kiu = const.tile([2, W2], F32R, tag="kiu")
        dma(kiu[:], ki_d.ap()[256:258])
        dsw_t = const.tile([128, 2, HA], F32R, tag="dsw")
        dma(dsw_t[:], dsw_d.ap().rearrange("(a p) n -> p a n", p=128))
        if use_ds_bias:
            dsb_t = const.tile([2, HA], F32R, tag="dsb")
            dma(dsb_t[:], dsb_d.ap())
        ident = const.tile([128, 128], F32R, tag="ident")
        dma(ident[:], id_d.ap())
        maskm = const.tile([128, 128], BF16, tag="maskm")
        dma(maskm[:], mask_d.ap())
        keysT = const.tile([128, 4, W2], F32R, tag="keysT")
        vals = const.tile([128, 12, 8, 66], BF16, tag="vals")
        dma(vals[:, :, :, 64:66], vone_d.ap())
        dew_t = const.tile([128, 4, 256], F32R, tag="dew")
        dma(dew_t[:], dew_d.ap().rearrange("(a p) n -> p a n", p=128))
        onehot_t = const.tile([128, 4, R], F32, tag="onehot")
        dma(onehot_t[:], oh_d.ap())
        wv4 = const.tile([128, 4], F32, tag="wv4")
        dma(wv4[:], wv_d.ap())
        ones_col = const.tile([128, 1], F32R, tag="onescol")
        dma(ones_col[:], onec_d.ap())
        if use_de_bias:
            debc_t = const.tile([128, 256], F32, tag="debc")
            dma(debc_t[:], debc_d.ap())
        eps_t = const.tile([128, 1], F32, tag="eps")
        nc.gpsimd.memset(eps_t[:], EPS)

        mm = nc.tensor.matmul

        # evac engine rotation: DVE / ACT
        rot = [0]

        def cp(out_ap, in_ap):
            if rot[0] % 2 == 0:
                nc.vector.tensor_copy(out_ap, in_ap)
            else:
                nc.scalar.copy(out_ap, in_ap)
            rot[0] += 1

        def relu_ev(out_ap, in_ap, bias_ap):
            if rot[0] % 2 == 0:
                if bias_ap is None:
                    nc.vector.tensor_scalar_max(out_ap, in_ap, 0.0)
                else:
                    nc.vector.tensor_scalar(out_ap, in_ap, bias_ap, 0.0,
                                            op0=ALU.add, op1=ALU.max)
            else:
                if bias_ap is None:
                    nc.scalar.activation(out_ap, in_ap, AF.Relu)
                else:
                    nc.scalar.activation(out_ap, in_ap, AF.Relu, bias=bias_ap)
            rot[0] += 1

        def ln_apply(out4, in4, lnbc_t, which):
            """LayerNorm over free dim (HA) for 4 [128,HA] slabs."""
            for evt in range(4):
                st6 = sm.tile([128, 6], F32, tag="st6")
                nc.vector.bn_stats(st6[:], in4[:, evt, :])
                mvv = sm.tile([128, 2], F32, tag="mv")
                nc.vector.bn_aggr(mvv[:], st6[:])
                if _LN_SQRT:
                    sd = sm.tile([128, 1], F32, tag="lnv")
                    nc.scalar.activation(sd[:], mvv[:, 1:2], AF.Sqrt,
                                         bias=eps_t[:, 0:1])
                    rs = sm.tile([128, 1], F32, tag="rs")
                    nc.vector.reciprocal(rs[:], sd[:])
                else:
                    lnv = sm.tile([128, 1], F32, tag="lnv")
                    nc.scalar.activation(lnv[:], mvv[:, 1:2], AF.Ln,
                                         bias=eps_t[:, 0:1])
                    rs = sm.tile([128, 1], F32, tag="rs")
                    nc.scalar.activation(rs[:], lnv[:], AF.Exp, scale=-0.5)
                nb = sm.tile([128, 1], F32, tag="nb")
                nc.vector.tensor_scalar(nb[:], mvv[:, 0:1], rs[:, 0:1], -1.0,
                                        op0=ALU.mult, op1=ALU.mult)
                dst = out4[:, evt, :]
                if ln_affine:
                    tmp = sm.tile([128, HA], F32, tag="lntmp")
                    nc.scalar.activation(tmp[:], in4[:, evt, :], AF.Identity,
                                         bias=nb[:, 0:1], scale=rs[:, 0:1])
                    g = lnbc_t[:, which * 2, :]
                    b = lnbc_t[:, which * 2 + 1, :]
                    nc.vector.tensor_mul(tmp[:], tmp[:], g)
                    nc.vector.tensor_add(dst, tmp[:], b)
                elif evt % 2 == 0:
                    nc.vector.tensor_scalar(dst, in4[:, evt, :], rs[:, 0:1],
                                            nb[:, 0:1], op0=ALU.mult,
                                            op1=ALU.add)
                else:
                    nc.scalar.activation(dst, in4[:, evt, :], AF.Identity,
                                         bias=nb[:, 0:1], scale=rs[:, 0:1])

        def transpose_4(outT, c, src4):
            """src4 [128,4,512] natural -> outT[:, c, :] = [ha-chunk c, v-cols]."""
            p_tr = ps.tile([128, 512], F32R, tag="p1")
            for evt in range(4):
                nc.tensor.transpose(p_tr[:, ts(evt, 128)],
                                    src4[:, evt, ts(c, 128)], ident[:])
            cp(outT[:, c, :], p_tr[:])

        # ================== dimension-shifting init ==================
        att = atp.tile([128, 4, HA], F32R, tag="att")
        for evt in range(4):
            e, vt = divmod(evt, 2)
            kc = e * W + NH + vt * 128
            p = ps.tile([128, 512], F32, tag="p1")
            mm(p[:], ki0[:, kc:kc + 128], dsw_t[:, 0, :], start=True, stop=False)
            last = not use_ds_bias
            mm(p[:], ki1[:, kc:kc + 128], dsw_t[:, 1, :], start=False, stop=last)
            if use_ds_bias:
                mm(p[:], kiu[0:2, kc:kc + 128], dsb_t[:, :], start=False, stop=True)
            cp(att[:, evt, :], p[:])
        attT = atT.tile([128, 4, 2 * NV], F32R, tag="attT")
        for t in range(4):
            p = ps.tile([128, 512], F32, tag="p1")
            for e in range(2):
                pc = e * W + NH
                reg = p[:, ts(e, 256)]
                mm(reg, dsw_t[:, 0, ts(t, 128)], ki0[:, pc:pc + 256],
                   start=True, stop=False)
                last = not use_ds_bias
                mm(reg, dsw_t[:, 1, ts(t, 128)], ki1[:, pc:pc + 256],
                   start=False, stop=last)
                if use_ds_bias:
                    mm(reg, dsb_t[:, ts(t, 128)], kiu[0:2, pc:pc + 256],
                       start=False, stop=True)
            cp(attT[:, t, :], p[:])
        if _DEBUG:
            dma(dbg_att0_d.ap()[:], att[:].bitcast(F32))
            dma(dbg_attT0_d.ap()[:], attT[:].bitcast(F32))

        # ================== layers ==================
        for l in range(L):
            kw_t = kvw.tile([128, 2, HA], F32R, tag="kw")
            dma(kw_t[:], kw_d.ap()[l].rearrange("(a p) n -> p a n", p=128))
            vw_t = kvw.tile([128, 2, HA], F32R, tag="vw")
            dma(vw_t[:], vw_d.ap()[l].rearrange("(a p) n -> p a n", p=128))
            kvu_t = kvw.tile([2, 2, HA], F32R, tag="kvu")
            dma(kvu_t[:], kvu_d.ap()[l])
            f1_t = ffw.tile([128, 4, M], F32R, tag="f1")
            dma(f1_t[:], f1_d.ap()[l].rearrange("(a p) n -> p a n", p=128))
            f2_t = ffw.tile([128, 4, M], F32R, tag="f2")
            dma(f2_t[:], f2_d.ap()[l].rearrange("(a p) n -> p a n", p=128))
            f3_t = ffw.tile([128, 4, HA], F32R, tag="f3")
            dma(f3_t[:], f3_d.ap()[l].rearrange("(a p) n -> p a n", p=128))
            if use_ff_bias:
                fbc_t = ffw.tile([128, 12], F32, tag="fbc")
                dma(fbc_t[:], fbc_d.ap()[l])
            if ln_affine:
                lnbc_t = ffw.tile([128, 4, HA], F32, tag="lnbc")
                dma(lnbc_t[:], lnbc_d.ap()[l])
            else:
                lnbc_t = None

            # ---- keysT [ha, w] both elems ----
            for t in range(4):
                p = ps.tile([128, W2], F32, tag="sc")
                for c in range(3):
                    reg = p[:, ts(c, 512)]
                    mm(reg, kw_t[:, 0, ts(t, 128)], ki0[:, ts(c, 512)],
                       start=True, stop=False)
                    mm(reg, kw_t[:, 1, ts(t, 128)], ki1[:, ts(c, 512)],
                       start=False, stop=False)
                    nk = 2 if use_kv_bias else 1
                    mm(reg, kvu_t[0:nk, 0, ts(t, 128)], kiu[0:nk, ts(c, 512)],
                       start=False, stop=True)
                cp(keysT[:, t, :], p[:])

            # ---- vals [w, (h,a)] bf16, both elems ----
            for g in range(4):
                p = ps.tile([128, W2], F32, tag="sc")
                for c in range(3):
                    ew = g * 3 + c
                    wlo = (ew // 6) * W + (ew % 6) * 128
                    reg = p[:, ts(c, 512)]
                    mm(reg, ki0[:, wlo:wlo + 128], vw_t[:, 0, :],
                       start=True, stop=False)
                    mm(reg, ki1[:, wlo:wlo + 128], vw_t[:, 1, :],
                       start=False, stop=False)
                    nk = 2 if use_kv_bias else 1
                    mm(reg, kiu[0:nk, wlo:wlo + 128], kvu_t[0:nk, 1, :],
                       start=False, stop=True)
                cp(vals[:, g * 3:(g + 1) * 3, :, 0:64],
                   p[:].rearrange("p (c h a) -> p c h a", c=3, h=8))

            # ---- attention: 16 (head, elem) streams, pipelined ----
            att_res = atp.tile([128, 4, HA], F32R, tag="att")

            def finish_stream(exh):
                ex, h, e = exh
                p_av = ps.tile([128, 512], F32, tag="p1")
                for vc in range(2):
                    nw = 5 if vc == 0 else 6
                    reg = p_av[:, vc * 256:vc * 256 + 66]
                    for wt in range(nw):
                        mm(reg, ex[:, wt * 256 + vc * 128: wt * 256 + (vc + 1) * 128],
                           vals[:, e * 6 + wt, h, :],
                           start=(wt == 0), stop=(wt == nw - 1))
                rec = sm.tile([128, 2], F32, tag="rec")
                for vc in range(2):
                    nc.vector.reciprocal(rec[:, vc:vc + 1],
                                         p_av[:, vc * 256 + 64:vc * 256 + 65])
                for vc in range(2):
                    evt = e * 2 + vc
                    nc.vector.scalar_tensor_tensor(
                        att_res[:, evt, ts(h, 64)],
                        p_av[:, vc * 256:vc * 256 + 64],
                        rec[:, vc:vc + 1],
                        att[:, evt, ts(h, 64)],
                        op0=ALU.mult, op1=ALU.add)

            prev = None
            for i in range(16):
                h, e = i // 2, i % 2
                t, base = h // 2, (h % 2) * 64
                p_sc = ps.tile([128, W2], F32, tag="sc")
                for wt in range(6):
                    mm(p_sc[:, ts(wt, 256)],
                       keysT[base:base + 64, t, e * W + wt * 128:e * W + (wt + 1) * 128],
                       attT[base:base + 64, t, ts(e, 256)],
                       start=True, stop=True)
                ex = ep.tile([128, W2], BF16, tag="exp")
                nc.scalar.activation(ex[:], p_sc[:], AF.Exp, scale=SCALE)
                mask_eng = nc.vector if i % 2 == 0 else nc.gpsimd
                mask_eng.tensor_mul(ex[:, 1024:1152], ex[:, 1024:1152], maskm[:])
                mask_eng.tensor_mul(ex[:, 1408:1536], ex[:, 1408:1536], maskm[:])
                if prev is not None:
                    finish_stream(prev)
                if _DEBUG and l == 0 and i == 0:
                    dma(dbg_exp_d.ap()[:], ex[:])
                prev = (ex, h, e)
            finish_stream(prev)
            if _DEBUG and l == 0:
                dma(dbg_keysT_d.ap()[:], keysT[:].bitcast(F32))
                dma(dbg_vals_d.ap()[:], vals[:])
                dma(dbg_attres_d.ap()[:], att_res[:].bitcast(F32))

            # ---- LN1 + att1T ----
            att1 = atp.tile([128, 4, HA], F32R, tag="att")
            ln_apply(att1, att_res, lnbc_t, 0)
            if _DEBUG and l == 0:
                dma(dbg_att1_d.ap()[:], att1[:].bitcast(F32))
            att1T = atT.tile([128, 4, 2 * NV], F32R, tag="attT")
            for c in range(4):
                transpose_4(att1T, c, att1)

            # ---- FF ----
            ff1T = ffp.tile([128, 4, 512], F32R, tag="ffT")
            for mt in range(4):
                p = ps.tile([128, 512], F32, tag="p1")
                for c in range(4):
                    mm(p[:], f1_t[:, c, ts(mt, 128)], att1T[:, c, :],
                       start=(c == 0), stop=(c == 3))
                bias = fbc_t[:, mt:mt + 1] if use_ff_bias else None
                relu_ev(ff1T[:, mt, :], p[:], bias)
            ff2T = ffp.tile([128, 4, 512], F32R, tag="ffT")
            for mt in range(4):
                p = ps.tile([128, 512], F32, tag="p1")
                for c in range(4):
                    mm(p[:], f2_t[:, c, ts(mt, 128)], ff1T[:, c, :],
                       start=(c == 0), stop=(c == 3))
                bias = fbc_t[:, 4 + mt:5 + mt] if use_ff_bias else None
                relu_ev(ff2T[:, mt, :], p[:], bias)
            f3s = []
            for c in range(4):
                p = ps.tile([128, 512], F32, tag="p1")
                for k in range(4):
                    mm(p[:], f3_t[:, k, ts(c, 128)], ff2T[:, k, :],
                       start=(k == 0), stop=(k == 3))
                f3c = ffp.tile([128, 512], F32R, tag="f3s", bufs=4)
                if use_ff_bias:
                    nc.vector.tensor_scalar(f3c[:], p[:], fbc_t[:, 8 + c:9 + c],
                                            None, op0=ALU.add)
                else:
                    cp(f3c[:], p[:])
                f3s.append(f3c)
            att2_res = atp.tile([128, 4, HA], F32R, tag="att")
            for evt in range(4):
                p_tr = ps.tile([128, 512], F32R, tag="p1")
                for c in range(4):
                    nc.tensor.transpose(p_tr[:, ts(c, 128)],
                                        f3s[c][:, ts(evt, 128)], ident[:])
                nc.vector.tensor_add(att2_res[:, evt, :], p_tr[:],
                                     att1[:, evt, :])

            # ---- LN2 + att2T ----
            att2 = atp.tile([128, 4, HA], F32R, tag="att")
            ln_apply(att2, att2_res, lnbc_t, 1)
            att2T = atT.tile([128, 4, 2 * NV], F32R, tag="attT")
            for c in range(4):
                transpose_4(att2T, c, att2)
            att, attT = att2, att2T

        # ================== loss head ==================
        if _DEBUG:
            dma(dbg_attL_d.ap()[:], att[:].bitcast(F32))
        q4 = sm.tile([128, 4], F32R, tag="q4", bufs=1)
        for evt in range(4):
            p = ps.tile([128, 512], F32, tag="p1")
            for c in range(4):
                mm(p[:, 0:256], attT[:, c, ts(evt, 128)], dew_t[:, c, :],
                   start=(c == 0), stop=(c == 3))
            if use_de_bias:
                nc.vector.tensor_add(p[:, 0:256], p[:, 0:256], debc_t[:])
            if _DEBUG and evt == 0:
                lgdbg = sm.tile([128, 256], F32, tag="lgdbg", bufs=1)
                nc.vector.tensor_copy(lgdbg[:], p[:, 0:256])
                dma(dbg_lg_d.ap()[:], lgdbg[:])
            scr = sm.tile([128, R], F32, tag="scr", bufs=2)
            se = sm.tile([128, 1], F32, tag="se")
            nc.scalar.activation(scr[:], p[:, 0:R], AF.Exp, accum_out=se[:])
            lse = sm.tile([128, 1], F32, tag="lse")
            nc.scalar.activation(lse[:], se[:], AF.Ln)
            scr2 = sm.tile([128, R], F32, tag="scr2", bufs=2)
            pick = sm.tile([128, 1], F32, tag="pick")
            nc.vector.scalar_tensor_tensor(scr2[:], p[:, 0:R], 1.0,
                                           onehot_t[:, evt, :],
                                           op0=ALU.mult, op1=ALU.mult,
                                           accum_out=pick[:])
            nc.vector.tensor_sub(q4[:, evt:evt + 1], pick[:], lse[:])
        nc.vector.tensor_mul(q4[:], q4[:], wv4[:])
        p = ps.tile([128, 512], F32, tag="p1")
        mm(p[0:1, 0:4], ones_col[:], q4[:], start=True, stop=True)
        res_sb = sm.tile([1, EPC], F32, tag="res", bufs=1)
        fbias = -(NV - 1) * math.log(R)
        for e in range(EPC):
            tot = sm.tile([1, 1], F32, tag="tot")
            nc.vector.tensor_reduce(tot[:], p[0:1, e * 2:e * 2 + 2],
                                    mybir.AxisListType.X, ALU.add)
            nc.vector.tensor_scalar(res_sb[0:1, e:e + 1], tot[:], -1.0, fbias,
                                    op0=ALU.mult, op1=ALU.add)
        dma(out_d.ap()[0:1, :], res_sb[:])

    nc.finalize()
    return nc


def _prep_inputs(inputs):
    import ml_dtypes
    bf = ml_dtypes.bfloat16

    def f(k):
        return np.asarray(inputs[k], np.float32)

    hist, hu = f("hist_encoded"), f("hist_true_u")
    pred, pu = f("pred_encoded"), f("pred_true_u")
    key_w, key_b = f("key_w"), f("key_b")
    val_w, val_b = f("val_w"), f("val_b")
    ds_w, ds_b = f("ds_w"), f("ds_b")
    ff_w1, ff_b1 = f("ff_w1"), f("ff_b1")
    ff_w2, ff_b2 = f("ff_w2"), f("ff_b2")
    ff_w3, ff_b3 = f("ff_w3"), f("ff_b3")
    de_w, de_b = f("de_w"), f("de_b")
    ln1_g, ln1_b = f("ln1_g"), f("ln1_b")
    ln2_g, ln2_b = f("ln2_g"), f("ln2_b")

    enc = np.concatenate([hist, pred], axis=1)        # [B, W, D]
    uu = np.concatenate([hu, pu], axis=1)             # [B, W]
    kiT = np.empty((B, 258, W), np.float32)
    kiT[:, 0:256, :] = enc.transpose(0, 2, 1)
    kiT[:, 256, :] = uu
    kiT[:, 257, :] = 1.0

    kwp = key_w[:, :, 0:256, :].transpose(0, 2, 1, 3).reshape(L, 256, HA)
    vwp = val_w[:, :, 0:256, :].transpose(0, 2, 1, 3).reshape(L, 256, HA)
    # device tile kvu_t is [row (u/ones) partitions, kv, HA]
    kvup = np.empty((L, 2, 2, HA), np.float32)
    kvup[:, 0, 0, :] = key_w[:, :, 256, :].reshape(L, HA)
    kvup[:, 0, 1, :] = val_w[:, :, 256, :].reshape(L, HA)
    kvup[:, 1, 0, :] = key_b.reshape(L, HA)
    kvup[:, 1, 1, :] = val_b.reshape(L, HA)

    dewp = np.zeros((512, 256), np.float32)
    dewp[:, 0:R] = de_w

    rho = np.arange(128)[:, None]
    vv = np.arange(128)[None, :]
    maskm = (vv > rho).astype(bf)

    tgt = np.clip(np.floor(pu * R).astype(np.int64), 0, R - 1)  # [B, NV]
    oh_elem = np.zeros((B, 2, 128, R), np.float32)
    for vt in range(2):
        idx = tgt[:, vt * 128:(vt + 1) * 128]
        oh_elem[np.arange(B)[:, None], vt, np.arange(128)[None, :], idx] = 1.0
    oh_elem[:, 0, 0, :] = 0.0  # v=0 excluded

    wv4 = np.ones((128, 4), np.float32)
    wv4[0, 0] = 0.0
    wv4[0, 2] = 0.0

    ident = np.eye(128, dtype=np.float32)
    vones = np.zeros((128, 12, 8, 2), np.float32)
    vones[:, :, :, 0] = 1.0

    use_ff_bias = bool(np.any(ff_b1) or np.any(ff_b2) or np.any(ff_b3))
    use_de_bias = bool(np.any(de_b))
    ln_affine = bool(np.any(ln1_g != 1.0) or np.any(ln1_b) or
                     np.any(ln2_g != 1.0) or np.any(ln2_b))
    use_ds_bias = bool(np.any(ds_b))
    use_kv_bias = bool(np.any(key_b) or np.any(val_b))

    shared = {
        "dswp": ds_w, "kwp": kwp, "vwp": vwp, "kvup": kvup,
        "ffw1": ff_w1, "ffw2": ff_w2, "ffw3": ff_w3,
        "dewp": dewp, "maskm": maskm, "ident": ident,
        "wv4": wv4, "vones": vones.astype(bf),
        "onescol": np.ones((128, 1), np.float32),
    }
    if use_ds_bias:
        dsb2 = np.zeros((2, HA), np.float32)
        dsb2[1] = ds_b
        shared["dsb"] = dsb2
    if use_ff_bias:
        fbc = np.empty((L, 128, 12), np.float32)
        for j, bb in enumerate((ff_b1, ff_b2, ff_b3)):
            fbc[:, :, j * 4:(j + 1) * 4] = bb.reshape(L, 4, 128).transpose(0, 2, 1)
        shared["ffbc"] = fbc
    if use_de_bias:
        debc = np.zeros((128, 256), np.float32)
        debc[:, 0:R] = de_b[None, :].repeat(128, 0)
        shared["debc"] = debc
    if ln_affine:
        lnbc = np.stack([ln1_g, ln1_b, ln2_g, ln2_b], axis=1)  # [L,4,HA]
        # device tile is [128, 4, HA]
        shared["lnbc"] = np.repeat(lnbc[:, None, :, :], 128, axis=1)

    in_maps = []
    for c in range(NCORES):
        m = dict(shared)
        kic = np.empty((258, W2), np.float32)
        ohc = np.empty((128, 4, R), np.float32)
        for e in range(EPC):
            be = c * EPC + e
            kic[:, e * W:(e + 1) * W] = kiT[be]
            ohc[:, e * 2:(e + 1) * 2, :] = oh_elem[be].swapaxes(0, 1)
        m["kiT"] = kic
        m["onehot"] = ohc
        in_maps.append(m)
    flags = (use_ff_bias, use_de_bias, ln_affine, use_ds_bias, use_kv_bias)
    return in_maps, flags


def _get_nc(flags):
    if flags not in _BUILD_CACHE:
        _BUILD_CACHE[flags] = _build(*flags)
    return _BUILD_CACHE[flags]


def _run(inputs, trace=False):
    from concourse.bass_utils import run_bass_kernel_spmd
    in_maps, flags = _prep_inputs(inputs)
    nc = _get_nc(flags)
    res = run_bass_kernel_spmd(nc, in_maps, list(range(NCORES)), trace=trace)
    out = np.concatenate([res.results[c]["out"].reshape(EPC)
                          for c in range(NCORES)])
    return out.astype(np.float32), res


def kernel(**inputs) -> np.ndarray:
    out, _ = _run(inputs, trace=False)
    return out
